# revision 33
# baseline (speedup 1.0000x reference)
"""Binarized Conv1d + BatchNorm1d (training mode) on 8 TRN2 NeuronCores.

Reference computation:
    bx  = sign(x)          [B=16, Cin=128, L=8192]
    bw  = sign(weight)     [Cout=128, Cin=128, K=5]
    out = conv1d(bx, bw, stride=1, pad=2) + bias
    out = (out - mean(out, (B,L))) * rsqrt(var(out, (B,L)) + 1e-5)

Sharding: data-parallel over batch, 2 batches per core.  Weights are
replicated.  Per-channel BN statistics are combined with a tiny
AllGather ([128,2] f32 per core: mean and E[x^2] of the local shard).

The conv bias cancels exactly inside training-mode BatchNorm
((conv + b) - mean(conv + b) == conv - mean(conv)), so it is ignored.

Kernel structure per core (v2 -- rebalanced from trace analysis):
  - dummy AllGather on garbage DRAM as the FIRST gpsimd instruction:
    the first collective pays a ~30 us cross-core rendezvous barrier +
    ~10 us CC-path warmup; triggering at t~9.6us (right after the Tile
    preamble) hides all of it under the conv phase so the real stats
    AllGather runs at its ~6 us steady-state floor
  - weights: DMA split over both HWDGE queues, sign on DVE (keeps ACT
    free for x sign), per-tap transpose via xbar DMA (frees all of
    PSUM and the PE for conv)
  - conv: PSUM as 2 quad tiles [128, 4, 512] (4 banks each); 20
    accumulated bf16 matmuls fill a quad while the other quad drains
    with ONE bn_stats [128,4,512]->[128,4,6] (DVE) and ONE 2048-col
    PSUM->SBUF copy (alternating ACT/DVE) -- 4x fewer drain
    instructions than per-bank draining, so the conv phase is
    PE-bound instead of ACT-bound
  - sign(x) -> bf16 padded row [128, 8196] on ACT, chunk sizes ramped
    so the first matmuls start early
  - stats: bn_aggr -> pack (mean, E[x^2]) -> AllGather -> ONE gather
    DMA [128, 8, 2] -> strided reduce -> compact rstd/shift chain
  - normalize: 16 x 1024-col chunks spread over DVE/ACT/GpSimd, each
    chunk's store DMA issued as soon as it is normalized
"""

import os
import sys

import numpy as np

# concourse is normally importable from the axon site; fall back to the
# staged repo copies if not
try:
    import concourse  # noqa: F401
except ImportError:
    for _p in ("/opt/trn_rl_repo", "/root/.axon_site/_ro/trn_rl_repo"):
        if os.path.isdir(_p):
            sys.path.insert(0, _p)
            break

B = 16
B_LOC = 2
CI = 128
CO = 128
L = 8192
K = 5
PAD = 2
EPS = 1e-5
N_CORES = 8
FREE = 512          # PSUM bank free dim (f32)
QUAD = 4            # PSUM banks per drain group
QFREE = QUAD * FREE # 2048 cols per drain
NQ = L // QFREE     # 4 quad groups per batch row
XCH = 1024          # out DMA chunk columns (512 KiB per transfer)

_CACHE = {}


def _build_nc():
    import concourse.bacc as bacc
    import concourse.bass as bass
    import concourse.tile as tile
    from concourse import mybir
    from concourse.masks import make_identity

    f32 = mybir.dt.float32
    bf16 = mybir.dt.bfloat16
    Sign = mybir.ActivationFunctionType.Sign
    Rsqrt = mybir.ActivationFunctionType.Rsqrt
    Copy = mybir.ActivationFunctionType.Copy
    Ident = mybir.ActivationFunctionType.Identity

    nc = bacc.Bacc("TRN2", target_bir_lowering=False, debug=False, num_devices=N_CORES)

    x = nc.declare_dram_parameter("x", [B_LOC, CI, L], f32, isOutput=False)
    w = nc.declare_dram_parameter("weight", [CO, CI, K], f32, isOutput=False)
    out = nc.declare_dram_parameter("out", [B_LOC, CO, L], f32, isOutput=True)

    with tile.TileContext(nc) as tc:
        with (
            tc.tile_pool(name="singles", bufs=1) as singles,
            tc.tile_pool(name="xin", bufs=1) as xin,
            tc.tile_pool(name="bxp", bufs=2) as bxp_pool,
            tc.tile_pool(name="dram", bufs=2, space="DRAM") as dram,
        ):
            # ---- warm-up collective: the very first gpsimd instruction ----
            # Contents are irrelevant (bypass op, output unused), so no
            # memset / staging DMA: the trigger has zero dependencies and
            # fires the moment the Tile preamble ends.  The first collective
            # pays a ~42 us rendezvous-barrier + CC-warmup chain and the CC
            # stream only frees up ~62 us after the trigger, so triggering
            # at ~7 us (vs ~14 us with input deps) moves the whole chain as
            # early as it can go; the real stats AllGather behind it on the
            # in-order CC stream then starts at the earliest possible time.
            warm_in = dram.tile([1, 8], f32)
            warm_out = dram.tile([N_CORES, 8], f32)
            nc.gpsimd.collective_compute(
                "AllGather",
                mybir.AluOpType.bypass,
                replica_groups=[list(range(N_CORES))],
                ins=[warm_in[:].opt()],
                outs=[warm_out[:].opt()],
            )

            # ---- weight + first x chunk DMAs split over both HWDGE queues ----
            wf32 = singles.tile([CO, CI, K], f32)
            nc.sync.dma_start(out=wf32[:, 0:64, :], in_=w[:, 0:64, :])
            nc.scalar.dma_start(out=wf32[:, 64:128, :], in_=w[:, 64:128, :])
            xts = []
            for b in range(B_LOC):
                xts.append(xin.tile([CI, L], f32, tag=f"xt{b}", name=f"xt{b}"))

            # ramped x chunks: small first chunks so the first sign +
            # matmuls start early; all on the sync queue (the scalar
            # engine's queue would stall issues behind ACT compute)
            CHUNK_SCHED = [
                [512, 512, 1024, 2048, 2048, 2048],
                [2048, 2048, 2048, 1024, 512, 512],
            ]
            off = 0
            for ch in CHUNK_SCHED[0]:
                nc.sync.dma_start(
                    out=xts[0][:, off : off + ch], in_=x[0, :, off : off + ch]
                )
                off += ch
            off = 0
            for ch in CHUNK_SCHED[1]:
                nc.sync.dma_start(
                    out=xts[1][:, off : off + ch], in_=x[1, :, off : off + ch]
                )
                off += ch

            # ---- weights: sign -> bf16, PE-transpose each tap ----
            # transposes run in a short-lived 1-bank PSUM pool that closes
            # before the conv quad pool opens, so conv still gets all 8
            # PSUM banks
            wsgn = singles.tile([CO, CI, K], bf16)
            nc.scalar.activation(out=wsgn, in_=wf32, func=Sign)
            wT = singles.tile([CI, K, CO], bf16)  # stationary tiles per tap
            ident = singles.tile([128, 128], bf16)
            make_identity(nc, ident)
            with tc.tile_pool(name="ptr", bufs=1, space="PSUM") as ptr_pool:
                for k in range(K):
                    pw = ptr_pool.tile([CI, CO], bf16, tag="tr")
                    nc.tensor.transpose(pw, wsgn[:, :, k], ident)
                    nc.vector.tensor_copy(out=wT[:, k, :], in_=pw)

            # ---- conv + local stats ----
            # conv output kept resident in SBUF: [128 co, B_LOC * L] f32
            conv_sb = singles.tile([CO, B_LOC, L], f32)
            stats = singles.tile([CO, B_LOC * NQ, QUAD, 6], f32)

            # drain-copy engine schedule (ACT also does all the sign work;
            # DVE does all bn_stats): the last quad goes to ACT so bn_aggr
            # is not stuck behind a 2 us DVE copy in the DVE FIFO
            COPY_ENG = [0, 1, 0, 1, 1, 0, 1, 1]  # 0=DVE tensor_copy, 1=ACT copy

            with tc.tile_pool(name="psum", bufs=2, space="PSUM") as psum:
                for b in range(B_LOC):
                    bxp = bxp_pool.tile([CI, L + 2 * PAD], bf16)
                    nc.vector.memset(bxp[:, 0:PAD], 0.0)
                    nc.vector.memset(bxp[:, L + PAD : L + 2 * PAD], 0.0)
                    xt = xts[b]
                    off = 0
                    for ch in CHUNK_SCHED[b]:
                        s = off
                        while s < off + ch:
                            sw = min(1024, off + ch - s)
                            nc.scalar.activation(
                                out=bxp[:, PAD + s : PAD + s + sw],
                                in_=xt[:, s : s + sw],
                                func=Sign,
                            )
                            s += sw
                        off += ch
                    for q in range(NQ):
                        pt = psum.tile([CO, QUAD, FREE], f32, tag="pt")
                        for j in range(QUAD):
                            t = q * QUAD + j
                            for k in range(K):
                                nc.tensor.matmul(
                                    pt[:, j, :],
                                    lhsT=wT[:, k, :],
                                    rhs=bxp[:, t * FREE + k : t * FREE + k + FREE],
                                    start=(k == 0),
                                    stop=(k == K - 1),
                                )
                        g = b * NQ + q
                        for j in range(QUAD):
                            nc.vector.bn_stats(out=stats[:, g, j, :], in_=pt[:, j, :])
                        dst = conv_sb[:, b, q * QFREE : (q + 1) * QFREE]
                        if COPY_ENG[g] == 0:
                            nc.vector.tensor_copy(out=dst, in_=pt)
                        else:
                            nc.scalar.activation(out=dst, in_=pt, func=Copy)

            # ---- global stats: AllGather (mean, E[x^2]) sums ----
            # bn_aggr writes (mean, var); turn the var slot into E[x^2];
            # the /N_CORES is folded into the post-AG chain
            pk = singles.tile([CO, 2], f32)
            sq = singles.tile([CO, 1], f32)
            nc.vector.bn_aggr(out=pk, in_=stats.rearrange("p g q c -> p (g q) c"))
            nc.vector.tensor_mul(sq, pk[:, 0:1], pk[:, 0:1])
            nc.vector.tensor_add(pk[:, 1:2], pk[:, 1:2], sq)

            cc_in = dram.tile([CO, 2], f32)
            cc_out = dram.tile([N_CORES * CO, 2], f32)
            nc.sync.dma_start(out=cc_in, in_=pk)
            nc.gpsimd.collective_compute(
                "AllGather",
                mybir.AluOpType.bypass,
                replica_groups=[list(range(N_CORES))],
                ins=[cc_in[:].opt()],
                outs=[cc_out[:].opt()],
            )
            # one gather DMA: [8*CO, 2] dram -> [CO, 8, 2] sbuf
            gsum = singles.tile([CO, N_CORES, 2], f32)
            nc.sync.dma_start(
                out=gsum, in_=cc_out.rearrange("(r p) c -> p r c", p=CO)
            )
            gst = singles.tile([CO, 2], f32)
            nc.vector.reduce_sum(
                out=gst,
                in_=gsum.rearrange("p r c -> p c r"),
                axis=mybir.AxisListType.X,
            )

            # gmean = sum/8 ; gvar = E2sum/8 - gmean^2
            # rstd = rsqrt(gvar + eps) ; shift = -gmean*rstd
            gmean = singles.tile([CO, 1], f32)
            gm2 = singles.tile([CO, 1], f32)
            gvar = singles.tile([CO, 1], f32)
            sd = singles.tile([CO, 1], f32)
            rstd = singles.tile([CO, 1], f32)
            shift = singles.tile([CO, 1], f32)
            eps_t = singles.tile([CO, 1], f32)
            nc.vector.memset(eps_t, EPS)
            nc.vector.tensor_scalar_mul(gmean, gst[:, 0:1], 1.0 / N_CORES)
            nc.vector.tensor_mul(gm2, gmean, gmean)
            nc.vector.tensor_scalar(
                out=gvar,
                in0=gst[:, 1:2],
                scalar1=1.0 / N_CORES,
                scalar2=gm2[:, 0:1],
                op0=mybir.AluOpType.mult,
                op1=mybir.AluOpType.subtract,
            )
            Sqrt = mybir.ActivationFunctionType.Sqrt
            nc.scalar.activation(out=sd, in_=gvar, func=Sqrt, bias=eps_t[:, 0:1])
            nc.vector.reciprocal(rstd, sd)
            nc.vector.tensor_scalar(
                out=shift,
                in0=gmean,
                scalar1=rstd[:, 0:1],
                scalar2=-1.0,
                op0=mybir.AluOpType.mult,
                op1=mybir.AluOpType.mult,
            )

            # ---- normalize (in place) + store ----
            # spread x*rstd+shift over DVE / ACT / GpSimd; each chunk's
            # store DMA is issued right behind its normalize.  ACT chunks
            # issue their own store on the scalar HWDGE queue (program
            # order, no cross-engine wait); the rest go via sync.
            ENG_SCHED = [0, 1, 2, 0, 1, 0, 0, 1, 2, 0, 1, 0, 0, 1, 2, 0]
            idx = 0
            for b in range(B_LOC):
                for c in range(L // XCH):
                    sl = conv_sb[:, b, c * XCH : (c + 1) * XCH]
                    eng = ENG_SCHED[idx % len(ENG_SCHED)]
                    if eng == 0:
                        nc.vector.tensor_scalar(
                            out=sl,
                            in0=sl,
                            scalar1=rstd[:, 0:1],
                            scalar2=shift[:, 0:1],
                            op0=mybir.AluOpType.mult,
                            op1=mybir.AluOpType.add,
                        )
                    elif eng == 1:
                        nc.scalar.activation(
                            out=sl,
                            in_=sl,
                            func=Ident,
                            bias=shift[:, 0:1],
                            scale=rstd[:, 0:1],
                        )
                    else:
                        nc.gpsimd.tensor_scalar(
                            out=sl,
                            in0=sl,
                            scalar1=rstd[:, 0:1],
                            scalar2=shift[:, 0:1],
                            op0=mybir.AluOpType.mult,
                            op1=mybir.AluOpType.add,
                        )
                    deng = nc.scalar if eng == 1 else nc.sync
                    deng.dma_start(
                        out=out[b, :, c * XCH : (c + 1) * XCH], in_=sl
                    )
                    idx += 1

    nc.compile()
    return nc


def _run(inputs, trace=False):
    from concourse import bass_utils

    x = np.ascontiguousarray(np.asarray(inputs["x"], dtype=np.float32))
    weight = np.ascontiguousarray(np.asarray(inputs["weight"], dtype=np.float32))

    if "nc" not in _CACHE:
        _CACHE["nc"] = _build_nc()
    nc = _CACHE["nc"]

    in_maps = [
        {"x": x[i * B_LOC : (i + 1) * B_LOC], "weight": weight}
        for i in range(N_CORES)
    ]
    res = bass_utils.run_bass_kernel_spmd(
        nc, in_maps, core_ids=list(range(N_CORES)), trace=trace
    )
    out = np.concatenate(
        [res.results[i]["out"] for i in range(N_CORES)], axis=0
    ).astype(np.float32)
    return out, res


def kernel(**inputs) -> np.ndarray:
    out, _ = _run(inputs, trace=False)
    return out


# revision 34
# speedup vs baseline: 1.1124x; 1.1124x over previous
"""Binarized Conv1d + BatchNorm1d (training mode) on 8 TRN2 NeuronCores.

Reference computation:
    bx  = sign(x)          [B=16, Cin=128, L=8192]
    bw  = sign(weight)     [Cout=128, Cin=128, K=5]
    out = conv1d(bx, bw, stride=1, pad=2) + bias
    out = (out - mean(out, (B,L))) * rsqrt(var(out, (B,L)) + 1e-5)

Sharding: data-parallel over batch, 2 batches per core.  Weights are
replicated.  Per-channel BN statistics are combined with a tiny
AllGather ([128,2] f32 per core: mean and E[x^2] of the local shard).

The conv bias cancels exactly inside training-mode BatchNorm
((conv + b) - mean(conv + b) == conv - mean(conv)), so it is ignored.

Kernel structure per core:
  - dummy AllGather on garbage DRAM as the FIRST gpsimd instruction:
    the first collective pays a ~40 us rendezvous barrier + ~20 us
    CC-path warmup, and the in-order CC stream frees up ~60 us after
    the trigger; triggering with zero dependencies right after the
    Tile preamble moves that whole chain as early as it can go
  - weights: DMA split over both HWDGE queues, sign -> bf16,
    PE-transpose each tap to [ci, co] stationary tiles
  - stream x in ramped chunks, sign -> bf16 padded row [128, 8196]
  - conv = 5 accumulated bf16 matmuls per [128, 512] PSUM tile
    (sign values are exact in bf16; products are +-1/0 accumulated in
    f32 PSUM, so the conv result is exact integers)
  - bn_stats on each PSUM tile (DVE), PSUM -> SBUF copy on ACT with
    every third tile's copy on DVE (balances ACT sign+copy work
    against DVE stats work; conv stays PE-bound)
  - bn_aggr -> pack (mean, E[x^2]) -> AllGather(bypass) -> ONE gather
    DMA [128, 8, 2] -> strided reduce -> rstd/shift chain
  - normalize in 1024-col chunks over DVE / ACT / GpSimd; each chunk's
    store DMA issues right behind its normalize (ACT chunks store on
    the scalar HWDGE queue, the rest on sync)
"""

import os
import sys

import numpy as np

# concourse is normally importable from the axon site; fall back to the
# staged repo copies if not
try:
    import concourse  # noqa: F401
except ImportError:
    for _p in ("/opt/trn_rl_repo", "/root/.axon_site/_ro/trn_rl_repo"):
        if os.path.isdir(_p):
            sys.path.insert(0, _p)
            break

B = 16
B_LOC = 2
CI = 128
CO = 128
L = 8192
K = 5
PAD = 2
EPS = 1e-5
N_CORES = 8
FREE = 512          # PSUM tile free dim (one bank of f32)
NT = L // FREE      # 16 conv tiles per batch row
XCH = 1024          # out DMA chunk columns (512 KiB per transfer)

_CACHE = {}


def _build_nc():
    import concourse.bacc as bacc
    import concourse.bass as bass
    import concourse.tile as tile
    from concourse import mybir
    from concourse.masks import make_identity

    f32 = mybir.dt.float32
    bf16 = mybir.dt.bfloat16
    Sign = mybir.ActivationFunctionType.Sign
    Sqrt = mybir.ActivationFunctionType.Sqrt
    Copy = mybir.ActivationFunctionType.Copy

    nc = bacc.Bacc("TRN2", target_bir_lowering=False, debug=False, num_devices=N_CORES)

    x = nc.declare_dram_parameter("x", [B_LOC, CI, L], f32, isOutput=False)
    w = nc.declare_dram_parameter("weight", [CO, CI, K], f32, isOutput=False)
    out = nc.declare_dram_parameter("out", [B_LOC, CO, L], f32, isOutput=True)

    with tile.TileContext(nc) as tc:
        with (
            tc.tile_pool(name="singles", bufs=1) as singles,
            tc.tile_pool(name="xin", bufs=1) as xin,
            tc.tile_pool(name="bxp", bufs=2) as bxp_pool,
            tc.tile_pool(name="psum", bufs=8, space="PSUM") as psum,
            tc.tile_pool(name="dram", bufs=2, space="DRAM") as dram,
        ):
            # ---- warm-up collective: the very first gpsimd instruction ----
            # Contents are irrelevant (bypass op, output unused), so no
            # memset / staging DMA: the trigger has zero dependencies and
            # fires the moment the Tile preamble ends.
            warm_in = dram.tile([1, 8], f32)
            warm_out = dram.tile([N_CORES, 8], f32)
            nc.gpsimd.collective_compute(
                "AllGather",
                mybir.AluOpType.bypass,
                replica_groups=[list(range(N_CORES))],
                ins=[warm_in[:].opt()],
                outs=[warm_out[:].opt()],
            )

            # ---- weight + first x chunk DMAs issued before anything else ----
            # weight halves on both HWDGE queues so sign(w) starts ~2 us
            # earlier than a single 330 KiB transfer would allow
            wf32 = singles.tile([CO, CI, K], f32)
            nc.sync.dma_start(out=wf32[:, 0:64, :], in_=w[:, 0:64, :])
            nc.scalar.dma_start(out=wf32[:, 64:128, :], in_=w[:, 64:128, :])
            xts = []
            for b in range(B_LOC):
                xts.append(
                    xin.tile([CI, L], f32, tag=f"xt{b}", name=f"xt{b}")
                )
            nc.sync.dma_start(out=xts[0][:, 0:512], in_=x[0, :, 0:512])

            # ---- weights: sign -> bf16, transpose each tap to [ci, co] ----
            ident = singles.tile([128, 128], bf16)
            make_identity(nc, ident)

            wsgn = singles.tile([CO, CI, K], bf16)
            nc.scalar.activation(out=wsgn, in_=wf32, func=Sign)

            wT = singles.tile([CI, K, CO], bf16)  # stationary tiles per tap
            for k in range(K):
                pw = psum.tile([CI, CO], bf16, tag="pt")
                nc.tensor.transpose(pw, wsgn[:, :, k], ident)
                nc.vector.tensor_copy(out=wT[:, k, :], in_=pw)

            # ---- conv + local stats ----
            # conv output kept resident in SBUF: [128 co, B_LOC * L] f32
            conv_sb = singles.tile([CO, B_LOC, L], f32)
            stats = singles.tile([CO, B_LOC * NT, 6], f32)

            # ramped DMA chunks: small first chunk (already issued above
            # for b=0) so the first matmuls start early, small last chunks
            # so the stats finish right behind the last sign; sign emitted
            # per <=1024 cols so matmuls chase the conversion closely
            CHUNK_SCHED = [
                [512, 512, 1024, 2048, 2048, 2048],
                [2048, 2048, 2048, 1024, 512, 512],
            ]
            for b in range(B_LOC):
                bxp = bxp_pool.tile([CI, L + 2 * PAD], bf16)
                nc.vector.memset(bxp[:, 0:PAD], 0.0)
                nc.vector.memset(bxp[:, L + PAD : L + 2 * PAD], 0.0)
                # one staging tile per batch, written once in disjoint
                # chunks -> no DMA ever needs a buffer-reuse wait (HW-queue
                # DMAs only support a single sync wait)
                xt = xts[b]
                off = 0
                for ci_, ch in enumerate(CHUNK_SCHED[b]):
                    if not (b == 0 and ci_ == 0):  # first chunk pre-issued
                        nc.sync.dma_start(
                            out=xt[:, off : off + ch],
                            in_=x[b, :, off : off + ch],
                        )
                    s = off
                    while s < off + ch:
                        sw = min(1024, off + ch - s)
                        nc.scalar.activation(
                            out=bxp[:, PAD + s : PAD + s + sw],
                            in_=xt[:, s : s + sw],
                            func=Sign,
                        )
                        s += sw
                    off += ch
                for t in range(NT):
                    pt = psum.tile([CO, FREE], f32, tag="pt")
                    for k in range(K):
                        nc.tensor.matmul(
                            pt,
                            lhsT=wT[:, k, :],
                            rhs=bxp[:, t * FREE + k : t * FREE + k + FREE],
                            start=(k == 0),
                            stop=(k == K - 1),
                        )
                    nc.vector.bn_stats(out=stats[:, b * NT + t, :], in_=pt)
                    dst = conv_sb[:, b, t * FREE : (t + 1) * FREE]
                    # every 3rd copy on DVE: ACT would otherwise be the
                    # conv-phase bottleneck (sign + all 32 copies > PE time)
                    if t % 3 == 2:
                        nc.vector.tensor_copy(out=dst, in_=pt)
                    else:
                        nc.scalar.activation(out=dst, in_=pt, func=Copy)

            # ---- global stats: all-reduce (mean, E[x^2]) sums ----
            # bn_aggr writes (mean, var); turn the var slot into E[x^2] in
            # place; the /N_CORES is folded into the post-AR chain
            pk = singles.tile([CO, 2], f32)
            sq = singles.tile([CO, 1], f32)
            nc.vector.bn_aggr(out=pk, in_=stats)
            nc.vector.tensor_mul(sq, pk[:, 0:1], pk[:, 0:1])
            nc.vector.tensor_add(pk[:, 1:2], pk[:, 1:2], sq)

            # AllGather ([128,2] per core -> [8*128,2]) has a lower floor
            # than AllReduce; the 8-way sum is done locally on DVE
            cc_in = dram.tile([CO, 2], f32)
            cc_out = dram.tile([N_CORES * CO, 2], f32)
            nc.sync.dma_start(out=cc_in, in_=pk)
            nc.gpsimd.collective_compute(
                "AllGather",
                mybir.AluOpType.bypass,
                replica_groups=[list(range(N_CORES))],
                ins=[cc_in[:].opt()],
                outs=[cc_out[:].opt()],
            )
            # one gather DMA: [8*CO, 2] dram -> [CO, 8, 2] sbuf
            gsum = singles.tile([CO, N_CORES, 2], f32)
            nc.sync.dma_start(
                out=gsum, in_=cc_out.rearrange("(r p) c -> p r c", p=CO)
            )
            gst = singles.tile([CO, 2], f32)
            nc.vector.reduce_sum(
                out=gst,
                in_=gsum.rearrange("p r c -> p c r"),
                axis=mybir.AxisListType.X,
            )

            # gmean = sum/8 ; gvar = E2sum/8 - gmean^2
            # rstd = 1/sqrt(gvar + eps) ; shift = -gmean*rstd
            gmean = singles.tile([CO, 1], f32)
            gm2 = singles.tile([CO, 1], f32)
            gvar = singles.tile([CO, 1], f32)
            sd = singles.tile([CO, 1], f32)
            rstd = singles.tile([CO, 1], f32)
            shift = singles.tile([CO, 1], f32)
            eps_t = singles.tile([CO, 1], f32)
            nc.vector.memset(eps_t, EPS)
            nc.vector.tensor_scalar_mul(gmean, gst[:, 0:1], 1.0 / N_CORES)
            nc.vector.tensor_mul(gm2, gmean, gmean)
            nc.vector.tensor_scalar(
                out=gvar,
                in0=gst[:, 1:2],
                scalar1=1.0 / N_CORES,
                scalar2=gm2[:, 0:1],
                op0=mybir.AluOpType.mult,
                op1=mybir.AluOpType.subtract,
            )
            nc.scalar.activation(out=sd, in_=gvar, func=Sqrt, bias=eps_t[:, 0:1])
            nc.vector.reciprocal(rstd, sd)
            # shift = -gmean * rstd in one op
            nc.vector.tensor_scalar(
                out=shift,
                in0=gmean,
                scalar1=rstd[:, 0:1],
                scalar2=-1.0,
                op0=mybir.AluOpType.mult,
                op1=mybir.AluOpType.mult,
            )

            # ---- normalize (in place) + store ----
            # distribute the x*rstd+shift pass across DVE / ACT / GpSimd so
            # the store phase is DMA-bound instead of DVE-paced; each
            # chunk's store DMA issues right behind its normalize (ACT
            # chunks store on the scalar HWDGE queue, the rest on sync)
            Ident = mybir.ActivationFunctionType.Identity
            ENG_SCHED = [0, 1, 2, 0, 1, 0, 0, 1, 2, 0, 1, 0, 0, 1, 2, 0]
            idx = 0
            for b in range(B_LOC):
                for c in range(L // XCH):
                    sl = conv_sb[:, b, c * XCH : (c + 1) * XCH]
                    eng = ENG_SCHED[idx % len(ENG_SCHED)]
                    if eng == 0:
                        nc.vector.tensor_scalar(
                            out=sl,
                            in0=sl,
                            scalar1=rstd[:, 0:1],
                            scalar2=shift[:, 0:1],
                            op0=mybir.AluOpType.mult,
                            op1=mybir.AluOpType.add,
                        )
                    elif eng == 1:
                        nc.scalar.activation(
                            out=sl,
                            in_=sl,
                            func=Ident,
                            bias=shift[:, 0:1],
                            scale=rstd[:, 0:1],
                        )
                    else:
                        nc.gpsimd.tensor_scalar(
                            out=sl,
                            in0=sl,
                            scalar1=rstd[:, 0:1],
                            scalar2=shift[:, 0:1],
                            op0=mybir.AluOpType.mult,
                            op1=mybir.AluOpType.add,
                        )
                    deng = nc.scalar if eng == 1 else nc.sync
                    deng.dma_start(
                        out=out[b, :, c * XCH : (c + 1) * XCH], in_=sl
                    )
                    idx += 1

    nc.compile()
    return nc


def _run(inputs, trace=False):
    from concourse import bass_utils

    x = np.ascontiguousarray(np.asarray(inputs["x"], dtype=np.float32))
    weight = np.ascontiguousarray(np.asarray(inputs["weight"], dtype=np.float32))

    if "nc" not in _CACHE:
        _CACHE["nc"] = _build_nc()
    nc = _CACHE["nc"]

    in_maps = [
        {"x": x[i * B_LOC : (i + 1) * B_LOC], "weight": weight}
        for i in range(N_CORES)
    ]
    res = bass_utils.run_bass_kernel_spmd(
        nc, in_maps, core_ids=list(range(N_CORES)), trace=trace
    )
    out = np.concatenate(
        [res.results[i]["out"] for i in range(N_CORES)], axis=0
    ).astype(np.float32)
    return out, res


def kernel(**inputs) -> np.ndarray:
    out, _ = _run(inputs, trace=False)
    return out


# revision 40
# speedup vs baseline: 1.2515x; 1.1250x over previous
"""Binarized Conv1d + BatchNorm1d (training mode) on 8 TRN2 NeuronCores.

Reference computation:
    bx  = sign(x)          [B=16, Cin=128, L=8192]
    bw  = sign(weight)     [Cout=128, Cin=128, K=5]
    out = conv1d(bx, bw, stride=1, pad=2) + bias
    out = (out - mean(out, (B,L))) * rsqrt(var(out, (B,L)) + 1e-5)

Sharding: data-parallel over batch, 2 batches per core.  Weights are
replicated.  Per-channel BN statistics are combined with a tiny
AllGather ([128,2] f32 per core: mean and E[x^2] of the local shard).

The conv bias cancels exactly inside training-mode BatchNorm
((conv + b) - mean(conv + b) == conv - mean(conv)), so it is ignored.

Kernel structure per core:
  - dummy AllGather on garbage DRAM as the FIRST gpsimd instruction:
    the first collective pays a ~40 us rendezvous barrier + ~20 us
    CC-path warmup, and the in-order CC stream frees up ~60 us after
    the trigger; triggering with zero dependencies right after the
    Tile preamble moves that whole chain as early as it can go
  - weights: DMA split over both HWDGE queues, sign -> bf16,
    PE-transpose each tap to [ci, co] stationary tiles
  - stream x in ramped chunks, sign -> bf16 padded row [128, 8196]
  - conv = 5 accumulated bf16 matmuls per [128, 512] PSUM tile
    (sign values are exact in bf16; products are +-1/0 accumulated in
    f32 PSUM, so the conv result is exact integers)
  - bn_stats on each PSUM tile (DVE), PSUM -> SBUF copy on ACT with
    every third tile's copy on DVE (balances ACT sign+copy work
    against DVE stats work; conv stays PE-bound)
  - bn_aggr -> pack (mean, E[x^2]) -> AllGather(bypass) -> ONE gather
    DMA [128, 8, 2] -> strided reduce -> rstd/shift chain
  - normalize in 1024-col chunks over DVE / ACT / GpSimd; each chunk's
    store DMA issues right behind its normalize (ACT chunks store on
    the scalar HWDGE queue, the rest on sync)
"""

import os
import sys

import numpy as np

# concourse is normally importable from the axon site; fall back to the
# staged repo copies if not
try:
    import concourse  # noqa: F401
except ImportError:
    for _p in ("/opt/trn_rl_repo", "/root/.axon_site/_ro/trn_rl_repo"):
        if os.path.isdir(_p):
            sys.path.insert(0, _p)
            break

B = 16
B_LOC = 2
CI = 128
CO = 128
L = 8192
K = 5
PAD = 2
EPS = 1e-5
N_CORES = 8
FREE = 512          # PSUM tile free dim (one bank of f32)
NT = L // FREE      # 16 conv tiles per batch row
XCH = 1024          # out DMA chunk columns (512 KiB per transfer)

_CACHE = {}


def _build_nc():
    import concourse.bacc as bacc
    import concourse.bass as bass
    import concourse.tile as tile
    from concourse import mybir
    from concourse.masks import make_identity

    f32 = mybir.dt.float32
    bf16 = mybir.dt.bfloat16
    Sign = mybir.ActivationFunctionType.Sign
    Sqrt = mybir.ActivationFunctionType.Sqrt
    Copy = mybir.ActivationFunctionType.Copy

    nc = bacc.Bacc("TRN2", target_bir_lowering=False, debug=False, num_devices=N_CORES)

    x = nc.declare_dram_parameter("x", [B_LOC, CI, L], f32, isOutput=False)
    w = nc.declare_dram_parameter("weight", [CO, CI, K], f32, isOutput=False)
    out = nc.declare_dram_parameter("out", [B_LOC, CO, L], f32, isOutput=True)

    with tile.TileContext(nc) as tc:
        with (
            tc.tile_pool(name="singles", bufs=1) as singles,
            tc.tile_pool(name="xin", bufs=1) as xin,
            tc.tile_pool(name="bxp", bufs=2) as bxp_pool,
            tc.tile_pool(name="psum", bufs=8, space="PSUM") as psum,
            tc.tile_pool(name="dram", bufs=2, space="DRAM") as dram,
        ):
            # ---- warm-up collective: the very first gpsimd instruction ----
            # Contents are irrelevant (bypass op, output unused), so no
            # memset / staging DMA: the trigger has zero dependencies and
            # fires the moment the Tile preamble ends.
            warm_in = dram.tile([1, 8], f32)
            warm_out = dram.tile([N_CORES, 8], f32)
            nc.gpsimd.collective_compute(
                "AllGather",
                mybir.AluOpType.bypass,
                replica_groups=[list(range(N_CORES))],
                ins=[warm_in[:].opt()],
                outs=[warm_out[:].opt()],
            )

            # ---- weight + first x chunk DMAs issued before anything else ----
            # weight halves on both HWDGE queues so sign(w) starts ~2 us
            # earlier than a single 330 KiB transfer would allow
            wf32 = singles.tile([CO, CI, K], f32)
            nc.sync.dma_start(out=wf32[:, 0:64, :], in_=w[:, 0:64, :])
            nc.scalar.dma_start(out=wf32[:, 64:128, :], in_=w[:, 64:128, :])
            xts = []
            for b in range(B_LOC):
                xts.append(
                    xin.tile([CI, L], f32, tag=f"xt{b}", name=f"xt{b}")
                )
            nc.sync.dma_start(out=xts[0][:, 0:512], in_=x[0, :, 0:512])

            # ---- weights: sign -> bf16, transpose each tap to [ci, co] ----
            ident = singles.tile([128, 128], bf16)
            make_identity(nc, ident)

            wsgn = singles.tile([CO, CI, K], bf16)
            nc.scalar.activation(out=wsgn, in_=wf32, func=Sign)

            # dummy matmuls on an uninitialized SBUF tile (values are
            # irrelevant, the PSUM bank is overwritten later): zero
            # dependencies, so they start the moment the Tile preamble
            # ends.  The PE's HAM clock gate needs ~3.4 us of sustained
            # activity to lift the 1.2 GHz cold throttle -- warm it up
            # before the first real matmul instead of paying the
            # half-clock ramp on real work.
            warm_mm = singles.tile([128, 128], bf16)
            nc.vector.memset(warm_mm, 0.0)
            warm_ps = psum.tile([128, FREE], f32, tag="pt")
            for _ in range(20):
                nc.tensor.matmul(
                    warm_ps[:, 0:128], lhsT=warm_mm, rhs=warm_mm,
                    start=True, stop=True,
                )

            wT = singles.tile([CI, K, CO], bf16)  # stationary tiles per tap
            for k in range(K):
                pw = psum.tile([CI, CO], bf16, tag="pt")
                nc.tensor.transpose(pw, wsgn[:, :, k], ident)
                nc.vector.tensor_copy(out=wT[:, k, :], in_=pw)

            # ---- conv + local stats ----
            # conv output kept resident in SBUF: [128 co, B_LOC * L] f32
            conv_sb = singles.tile([CO, B_LOC, L], f32)
            stats = singles.tile([CO, B_LOC * NT, 6], f32)

            # ramped DMA chunks: small first chunk (already issued above
            # for b=0) so the first matmuls start early, small last chunks
            # so the stats finish right behind the last sign; sign emitted
            # per <=1024 cols so matmuls chase the conversion closely
            CHUNK_SCHED = [
                [512, 512, 1024, 2048, 2048, 2048],
                [2048, 2048, 2048, 1024, 512, 512],
            ]
            for b in range(B_LOC):
                bxp = bxp_pool.tile([CI, L + 2 * PAD], bf16)
                nc.vector.memset(bxp[:, 0:PAD], 0.0)
                nc.vector.memset(bxp[:, L + PAD : L + 2 * PAD], 0.0)
                # one staging tile per batch, written once in disjoint
                # chunks -> no DMA ever needs a buffer-reuse wait (HW-queue
                # DMAs only support a single sync wait)
                xt = xts[b]
                off = 0
                for ci_, ch in enumerate(CHUNK_SCHED[b]):
                    if not (b == 0 and ci_ == 0):  # first chunk pre-issued
                        nc.sync.dma_start(
                            out=xt[:, off : off + ch],
                            in_=x[b, :, off : off + ch],
                        )
                    s = off
                    while s < off + ch:
                        sw = min(1024, off + ch - s)
                        nc.scalar.activation(
                            out=bxp[:, PAD + s : PAD + s + sw],
                            in_=xt[:, s : s + sw],
                            func=Sign,
                        )
                        s += sw
                    off += ch
                for t in range(NT):
                    pt = psum.tile([CO, FREE], f32, tag="pt")
                    for k in range(K):
                        nc.tensor.matmul(
                            pt,
                            lhsT=wT[:, k, :],
                            rhs=bxp[:, t * FREE + k : t * FREE + k + FREE],
                            start=(k == 0),
                            stop=(k == K - 1),
                        )
                    nc.vector.bn_stats(out=stats[:, b * NT + t, :], in_=pt)
                    dst = conv_sb[:, b, t * FREE : (t + 1) * FREE]
                    # every 3rd copy on DVE: ACT would otherwise be the
                    # conv-phase bottleneck (sign + all 32 copies > PE time)
                    if t % 3 == 2:
                        nc.vector.tensor_copy(out=dst, in_=pt)
                    else:
                        nc.scalar.activation(out=dst, in_=pt, func=Copy)

            # ---- global stats: all-reduce (mean, E[x^2]) sums ----
            # bn_aggr writes (mean, var); turn the var slot into E[x^2] in
            # place; the /N_CORES is folded into the post-AR chain
            pk = singles.tile([CO, 2], f32)
            sq = singles.tile([CO, 1], f32)
            nc.vector.bn_aggr(out=pk, in_=stats)
            nc.vector.tensor_mul(sq, pk[:, 0:1], pk[:, 0:1])
            nc.vector.tensor_add(pk[:, 1:2], pk[:, 1:2], sq)

            # AllGather ([128,2] per core -> [8*128,2]) has a lower floor
            # than AllReduce; the 8-way sum is done locally on DVE
            cc_in = dram.tile([CO, 2], f32)
            cc_out = dram.tile([N_CORES * CO, 2], f32)
            nc.sync.dma_start(out=cc_in, in_=pk)
            nc.gpsimd.collective_compute(
                "AllGather",
                mybir.AluOpType.bypass,
                replica_groups=[list(range(N_CORES))],
                ins=[cc_in[:].opt()],
                outs=[cc_out[:].opt()],
            )
            # one gather DMA: [8*CO, 2] dram -> [CO, 8, 2] sbuf
            gsum = singles.tile([CO, N_CORES, 2], f32)
            nc.sync.dma_start(
                out=gsum, in_=cc_out.rearrange("(r p) c -> p r c", p=CO)
            )
            gst = singles.tile([CO, 2], f32)
            nc.vector.reduce_sum(
                out=gst,
                in_=gsum.rearrange("p r c -> p c r"),
                axis=mybir.AxisListType.X,
            )

            # gvar = E2sum/8 - (sum/8)^2 ; rstd = 1/sqrt(gvar + eps)
            # shift = -(sum/8)*rstd = (sum * rstd) * (-1/8)
            gm2 = singles.tile([CO, 1], f32)
            gvar = singles.tile([CO, 1], f32)
            sd = singles.tile([CO, 1], f32)
            rstd = singles.tile([CO, 1], f32)
            shift = singles.tile([CO, 1], f32)
            eps_t = singles.tile([CO, 1], f32)
            nc.vector.memset(eps_t, EPS)
            nc.vector.tensor_scalar(
                out=gm2,
                in0=gst[:, 0:1],
                scalar1=gst[:, 0:1],
                scalar2=1.0 / (N_CORES * N_CORES),
                op0=mybir.AluOpType.mult,
                op1=mybir.AluOpType.mult,
            )
            nc.vector.tensor_scalar(
                out=gvar,
                in0=gst[:, 1:2],
                scalar1=1.0 / N_CORES,
                scalar2=gm2[:, 0:1],
                op0=mybir.AluOpType.mult,
                op1=mybir.AluOpType.subtract,
            )
            nc.scalar.activation(out=sd, in_=gvar, func=Sqrt, bias=eps_t[:, 0:1])
            nc.vector.reciprocal(rstd, sd)
            nc.vector.tensor_scalar(
                out=shift,
                in0=gst[:, 0:1],
                scalar1=rstd[:, 0:1],
                scalar2=-1.0 / N_CORES,
                op0=mybir.AluOpType.mult,
                op1=mybir.AluOpType.mult,
            )

            # ---- normalize (in place) + store ----
            # distribute the x*rstd+shift pass across DVE / ACT / GpSimd so
            # the store phase is DMA-bound instead of DVE-paced; each
            # chunk's store DMA issues right behind its normalize (ACT
            # chunks store on the scalar HWDGE queue, the rest on sync)
            Ident = mybir.ActivationFunctionType.Identity
            ENG_SCHED = [0, 1, 2, 0, 1, 0, 0, 1, 2, 0, 1, 0, 0, 1, 2, 0]
            idx = 0
            for b in range(B_LOC):
                for c in range(L // XCH):
                    sl = conv_sb[:, b, c * XCH : (c + 1) * XCH]
                    eng = ENG_SCHED[idx % len(ENG_SCHED)]
                    if eng == 0:
                        nc.vector.tensor_scalar(
                            out=sl,
                            in0=sl,
                            scalar1=rstd[:, 0:1],
                            scalar2=shift[:, 0:1],
                            op0=mybir.AluOpType.mult,
                            op1=mybir.AluOpType.add,
                        )
                    elif eng == 1:
                        nc.scalar.activation(
                            out=sl,
                            in_=sl,
                            func=Ident,
                            bias=shift[:, 0:1],
                            scale=rstd[:, 0:1],
                        )
                    else:
                        nc.gpsimd.tensor_scalar(
                            out=sl,
                            in0=sl,
                            scalar1=rstd[:, 0:1],
                            scalar2=shift[:, 0:1],
                            op0=mybir.AluOpType.mult,
                            op1=mybir.AluOpType.add,
                        )
                    # stores split across both HWDGE queues (DVE chunks ->
                    # sync, ACT/GpSimd chunks -> scalar) so the 8 MiB
                    # output drains at the two-queue rate
                    deng = nc.sync if eng == 0 else nc.scalar
                    deng.dma_start(
                        out=out[b, :, c * XCH : (c + 1) * XCH], in_=sl
                    )
                    idx += 1

    nc.compile()
    return nc


def _run(inputs, trace=False):
    from concourse import bass_utils

    x = np.ascontiguousarray(np.asarray(inputs["x"], dtype=np.float32))
    weight = np.ascontiguousarray(np.asarray(inputs["weight"], dtype=np.float32))

    if "nc" not in _CACHE:
        _CACHE["nc"] = _build_nc()
    nc = _CACHE["nc"]

    in_maps = [
        {"x": x[i * B_LOC : (i + 1) * B_LOC], "weight": weight}
        for i in range(N_CORES)
    ]
    res = bass_utils.run_bass_kernel_spmd(
        nc, in_maps, core_ids=list(range(N_CORES)), trace=trace
    )
    out = np.concatenate(
        [res.results[i]["out"] for i in range(N_CORES)], axis=0
    ).astype(np.float32)
    return out, res


def kernel(**inputs) -> np.ndarray:
    out, _ = _run(inputs, trace=False)
    return out
